# revision 23
# baseline (speedup 1.0000x reference)
"""EquiMultiHeadAttention on 8 Trainium2 NeuronCores.

Sharding: one attention head per core (H=8, n_cores=8). Each core computes,
for all 4 batches, its head's q/k/v projections, the full SxS attention, and
that head's contribution to the output projection. The host sums the 8
partial outputs and adds the output bias (scalar blade only) plus the
attention-invariant v-bias contribution (softmax weights sum to exactly 1,
so the per-head v bias commutes with attention and is applied on the host).

Math folded into per-head host-precomputed weights:
  - q is packed to the 8 surviving mv components of the PGA inner product,
    pre-scaled by 1/sqrt(32); k packed identically -> the score matmul is a
    plain K=128 contraction.
  - The output projection (W_out columns of this head) is applied to v
    *before* attention, so the attention's second matmul directly produces
    this head's output contribution. An extra all-ones column on v yields
    the softmax denominator in the same matmul.

Whole pipeline runs in bf16 (measured end-to-end rel err ~8e-3):
  - x is cast to bf16 on the Pool engine and transposed to [(c,x), s] layout
    by the DMA xbar transpose (one instruction per 512-token quad; no PE
    transposes, no PSUM->SBUF transpose copies).
  - All matmuls are bf16 (1 cycle/row on the PE).
  - Scores are exp'd by the scalar engine in 1024-element groups
    (4 i-blocks x 256 j) straight out of PSUM into bf16 SBUF tiles.
  - x for batch b+1 is loaded/cast/transposed during the first half of
    batch b's attention and projected during the second half.
"""

import sys
import os

sys.path.insert(0, "/opt/trn_rl_repo")

import numpy as np
import ml_dtypes

B, S, C, X = 4, 2048, 16, 16
H = 8
CX = C * X  # 256
SURV = [0, 2, 3, 4, 8, 9, 10, 14]  # mv components surviving <q, ~k>
SCALE = 1.0 / np.sqrt(32.0)
NCORES = 8
SB, JB, IB = 128, 256, 128  # s-tile, j-block, i-block sizes
NST, NJB, NIB = S // SB, S // JB, S // IB  # 16, 8, 16
GRP = 4  # i-blocks per exp group
NG = NIB // GRP  # 4 groups
NV = CX + 1  # 257: v columns + denominator ones column
NWALL = 1024  # packed weight image columns (wq 256 | wk 256 | wvp 512)

_COMPILED = None


def _head_weights(h, W_qkv, b_qkv, W_out):
    """Per-head block-diagonal weight construction (bf16 device weights)."""
    f32 = np.float32
    bf16 = ml_dtypes.bfloat16
    # row h*48 + c'*3 + p  (p: 0=q, 1=k, 2=v)
    Wh = W_qkv[h * 48 : (h + 1) * 48].reshape(C, 3, C)  # [c', p, c]
    bh = b_qkv[h * 48 : (h + 1) * 48].reshape(C, 3)  # [c', p]
    Wq, Wk, Wv = Wh[:, 0], Wh[:, 1], Wh[:, 2]  # each [c', c]
    qb, kb, vb = bh[:, 0], bh[:, 1], bh[:, 2]
    Wout_h = W_out[:, np.arange(C) * H + h]  # [o, c']
    Wvp = Wout_h @ Wv  # [o, c]
    vbp = Wout_h @ vb  # [o] -> host-side add

    # x_T row layout within half: r = (c - half*8)*16 + xi
    # packed q/k column layout: d = c'*8 + si  (si indexes SURV)
    Wq_bd = np.zeros((2, 128, 128), f32)
    Wk_bd = np.zeros((2, 128, 128), f32)
    Wvp_bd = np.zeros((2, 128, CX), f32)
    for half in range(2):
        for cl in range(8):
            c = half * 8 + cl
            for si, xs in enumerate(SURV):
                r = cl * 16 + xs
                Wq_bd[half, r, np.arange(C) * 8 + si] = SCALE * Wq[:, c]
                Wk_bd[half, r, np.arange(C) * 8 + si] = Wk[:, c]
            for xi in range(16):
                r = cl * 16 + xi
                Wvp_bd[half, r, np.arange(C) * 16 + xi] = Wvp[:, c]
    qb_col = np.zeros((128, 1), f32)
    kb_col = np.zeros((128, 1), f32)
    qb_col[np.arange(C) * 8, 0] = SCALE * qb  # si=0 <-> x component 0
    kb_col[np.arange(C) * 8, 0] = kb
    # single packed bf16 weight image: [wq(2x128) | wk(2x128) | wvp(2x256)]
    # plus a tiny f32 bias pair -> two DMAs instead of eight
    W_all = np.zeros((128, NWALL), f32)
    W_all[:, 0:256] = Wq_bd.transpose(1, 0, 2).reshape(128, 256)
    W_all[:, 256:512] = Wk_bd.transpose(1, 0, 2).reshape(128, 256)
    W_all[:, 512:1024] = Wvp_bd.transpose(1, 0, 2).reshape(128, 512)
    qkb = np.concatenate([qb_col, kb_col], axis=1)  # [128, 2] f32
    return {"W_all": W_all.astype(bf16), "qkb": qkb}, vbp


def _build_program():
    import concourse.bass as bass
    import concourse.mybir as mybir
    import concourse.tile as tile
    from concourse import bacc

    f32 = mybir.dt.float32
    bf16 = mybir.dt.bfloat16
    Exp = mybir.ActivationFunctionType.Exp

    nc = bacc.Bacc("TRN2", target_bir_lowering=False, debug=False)

    x_d = nc.dram_tensor("x", [B, S, CX], f32, kind="ExternalInput").ap()
    wall_d = nc.dram_tensor("W_all", [128, NWALL], bf16, kind="ExternalInput").ap()
    qkb_d = nc.dram_tensor("qkb", [128, 2], f32, kind="ExternalInput").ap()
    y_d = nc.dram_tensor("y", [B, S, CX], f32, kind="ExternalOutput").ap()

    with tile.TileContext(nc) as tc:
        with (
            tc.tile_pool(name="const", bufs=1) as const,
            tc.tile_pool(name="xin", bufs=8) as xin,
            tc.tile_pool(name="xtb", bufs=8) as xtbp,
            tc.tile_pool(name="xT", bufs=2) as xTp,
            tc.tile_pool(name="qk", bufs=2) as qkp,
            tc.tile_pool(name="vp", bufs=2) as vpp,
            tc.tile_pool(name="es", bufs=3) as esp,
            tc.tile_pool(name="yo", bufs=3) as yop,
            tc.tile_pool(name="psm", bufs=2, space="PSUM") as psm,
            tc.tile_pool(name="pss", bufs=2, space="PSUM") as pssp,
            tc.tile_pool(name="psy", bufs=1, space="PSUM") as psyp,
        ):
            state = {}

            def load_consts():
                wall = const.tile([128, NWALL], bf16, tag="wall", name="wall")
                nc.scalar.dma_start(out=wall[:], in_=wall_d[:])
                state["wq"] = [wall[:, h * 128 : (h + 1) * 128] for h in range(2)]
                state["wk"] = [wall[:, 256 + h * 128 : 256 + (h + 1) * 128] for h in range(2)]
                state["wvp"] = [wall[:, 512 + h * CX : 512 + (h + 1) * CX] for h in range(2)]
                qkb = const.tile([128, 2], f32, tag="qkb", name="qkb")
                nc.scalar.dma_start(out=qkb[:], in_=qkb_d[:])
                state["qb_sb"] = qkb[:, 0:1]
                state["kb_sb"] = qkb[:, 1:2]

            try:
                n_rep = int(os.environ.get("BASS_REPEAT", "1"))
            except ValueError:
                n_rep = 1

            def alloc_batch(b):
                # xT layout: [c-in-half, s-tile, half, s-within-tile]
                state["xT"] = xTp.tile([128, NST, 2, SB], bf16, tag="xT", name=f"xT{b}")
                state["qp"] = qkp.tile([128, S], bf16, tag="qp", name=f"qp{b}")
                state["kp"] = qkp.tile([128, S], bf16, tag="kp", name=f"kp{b}")
                vp = vpp.tile([128, NST, NV], bf16, tag="vp", name=f"vp{b}")
                # denominator ones column (the v bias itself is added on host)
                nc.gpsimd.memset(vp[:, :, CX : CX + 1], 1.0)
                state["vp"] = vp

            def load_quad(b, q):
                """DMA a 512-token quad of x and cast it to bf16 (Pool)."""
                xt = xin.tile([128, 4, CX], f32, tag="x", name="xt")
                src_ap = x_d[b, q * 512 : (q + 1) * 512, :].rearrange(
                    "(k p) c -> p k c", k=4, p=SB
                )
                nc.sync.dma_start(out=xt[:], in_=src_ap)
                xtb = xtbp.tile([128, 4, CX], bf16, tag="xtb", name="xtb")
                nc.gpsimd.tensor_copy(out=xtb[:], in_=xt[:])
                return xtb

            def transpose_quad(q, xtb):
                # out view [c, (st half), s] merges the st/half dims (contiguous)
                xT = state["xT"]
                dst = xT[:, 4 * q : 4 * q + 4].rearrange("c k h s -> c (k h) s")
                nc.sync.dma_start_transpose(
                    out=dst, in_=xtb[:].rearrange("p k c -> p (k c)")
                )

            def proj_quad(b, q):
                """Project one 512-token quad into qp/kp (bf16) and vp (bf16)."""
                wq, wk, wvp = state["wq"], state["wk"], state["wvp"]
                qb_sb, kb_sb = state["qb_sb"], state["kb_sb"]
                xT, qp, kp, vp = state["xT"], state["qp"], state["kp"], state["vp"]
                sl = slice(q * 512, (q + 1) * 512)
                stq = slice(4 * q, 4 * q + 4)
                pq = psm.tile([128, 512], f32, tag="misc", name="pq")
                nc.tensor.matmul(pq[:], wq[0], xT[:, stq, 0], start=True, stop=False)
                nc.tensor.matmul(pq[:], wq[1], xT[:, stq, 1], start=False, stop=True)
                nc.vector.tensor_scalar_add(out=qp[:, sl], in0=pq[:], scalar1=qb_sb)
                pk = psm.tile([128, 512], f32, tag="misc", name="pk")
                nc.tensor.matmul(pk[:], wk[0], xT[:, stq, 0], start=True, stop=False)
                nc.tensor.matmul(pk[:], wk[1], xT[:, stq, 1], start=False, stop=True)
                nc.vector.tensor_scalar_add(out=kp[:, sl], in0=pk[:], scalar1=kb_sb)
                for st2 in range(q * 2, q * 2 + 2):
                    pv = psm.tile([128, 512], f32, tag="misc", name="pv")
                    for u in range(2):
                        st = st2 * 2 + u
                        pvs = pv[:, u * 256 : (u + 1) * 256]
                        nc.tensor.matmul(pvs, xT[:, st, 0], wvp[0], start=True, stop=False)
                        nc.tensor.matmul(pvs, xT[:, st, 1], wvp[1], start=False, stop=True)
                        nc.vector.tensor_copy(out=vp[:, st, :CX], in_=pvs)

            def make_jb(b, qp, kp, vp, jb, tail=False):
                jsl = slice(jb * JB, (jb + 1) * JB)
                yps = [
                    psyp.tile([128, NV], f32, tag=f"yps{js}", name=f"yps{js}")
                    for js in range(2)
                ]
                es_q = {}

                def produce(g):
                    pss = pssp.tile([128, GRP, JB], f32, tag="ps_s", name="pss")
                    for gi in range(GRP):
                        ib = g * GRP + gi
                        isl = slice(ib * IB, (ib + 1) * IB)
                        nc.tensor.matmul(
                            pss[:, gi], kp[:, isl], qp[:, jsl], start=True, stop=True
                        )
                    es = esp.tile([128, GRP, JB], bf16, tag="es", name="es")
                    nc.scalar.activation(es[:, 0:2], pss[:, 0:2], Exp)
                    nc.scalar.activation(es[:, 2:4], pss[:, 2:4], Exp)
                    es_q[g] = es

                def consume(g):
                    es = es_q.pop(g)
                    for gi in range(GRP):
                        ib = g * GRP + gi
                        for js in range(2):
                            nc.tensor.matmul(
                                yps[js][:],
                                es[:, gi, js * 128 : (js + 1) * 128],
                                vp[:, ib],
                                start=(ib == 0),
                                stop=(ib == NIB - 1),
                            )

                def consume_last():
                    # js-major order so the js=0 normalization can start while
                    # the PE is still accumulating js=1; y stores go on the
                    # scalar HWDGE queue so they never backpressure x loads
                    es = es_q.pop(NG - 1)
                    ysb = yop.tile([128, 2, CX], f32, tag="ysb")
                    dst_ap = y_d[b, jb * JB : (jb + 1) * JB, :].rearrange(
                        "(k p) c -> p k c", k=2, p=SB
                    )
                    for js in range(2):
                        for gi in range(GRP):
                            ib = (NG - 1) * GRP + gi
                            nc.tensor.matmul(
                                yps[js][:],
                                es[:, gi, js * 128 : (js + 1) * 128],
                                vp[:, ib],
                                start=(ib == 0),
                                stop=(ib == NIB - 1),
                            )
                        rec = yop.tile([128, 1], f32, tag="rec")
                        nc.vector.reciprocal(rec[:], yps[js][:, CX : CX + 1])
                        nc.vector.tensor_scalar_mul(
                            out=ysb[:, js], in0=yps[js][:, :CX], scalar1=rec[:]
                        )
                        eng = nc.sync if tail else nc.gpsimd
                        eng.dma_start(out=dst_ap[:, js], in_=ysb[:, js])

                return produce, consume, consume_last

            def attend_jb(b, qp, kp, vp, jb, tail=False):
                produce, consume, consume_last = make_jb(b, qp, kp, vp, jb, tail)
                for g in range(NG):
                    produce(g)
                    if g >= 1 and g - 1 < NG - 1:
                        consume(g - 1)
                consume_last()

            for rep in range(n_rep):
                for b in range(B):
                    first = rep == 0 and b == 0
                    last = rep == n_rep - 1 and b == B - 1
                    if first:
                        # cold start: consts first, then quad 0 streamed at
                        # 128-token chunk granularity so the first projection
                        # matmuls start as early as possible
                        load_consts()
                        alloc_batch(b)
                        xT = state["xT"]
                        src0 = x_d[b, 0:512, :].rearrange("(k p) c -> p k c", k=4, p=SB)
                        xt0 = xin.tile([128, 4, CX], f32, tag="x", name="xt0")
                        xtb0 = xtbp.tile([128, 4, CX], bf16, tag="xtb", name="xtb0")
                        for k in range(4):
                            nc.sync.dma_start(out=xt0[:, k], in_=src0[:, k])
                        for k in range(4):
                            nc.gpsimd.tensor_copy(out=xtb0[:, k], in_=xt0[:, k])
                        # all x DMAs dispatched before any (cast-waiting)
                        # transpose: DMA waits block the whole SP queue
                        xtbs = [None] + [load_quad(b, q) for q in range(1, 4)]
                        for k in range(4):
                            nc.sync.dma_start_transpose(out=xT[:, k], in_=xtb0[:, k])
                        # chunked projections of quad 0 into shared pss tiles
                        wq, wk, wvp = state["wq"], state["wk"], state["wvp"]
                        qb_sb, kb_sb = state["qb_sb"], state["kb_sb"]
                        qp0, kp0, vp0 = state["qp"], state["kp"], state["vp"]
                        pq0 = psm.tile([128, 512], f32, tag="misc", name="pq0")
                        pk0 = psm.tile([128, 512], f32, tag="misc", name="pk0")
                        for k in range(4):
                            oq = pq0[:, k * 128 : (k + 1) * 128]
                            ok = pk0[:, k * 128 : (k + 1) * 128]
                            nc.tensor.matmul(oq, wq[0], xT[:, k, 0], start=True, stop=False)
                            nc.tensor.matmul(oq, wq[1], xT[:, k, 1], start=False, stop=True)
                            nc.tensor.matmul(ok, wk[0], xT[:, k, 0], start=True, stop=False)
                            nc.tensor.matmul(ok, wk[1], xT[:, k, 1], start=False, stop=True)
                        nc.vector.tensor_scalar_add(
                            out=qp0[:, 0:512], in0=pq0[:], scalar1=qb_sb
                        )
                        nc.vector.tensor_scalar_add(
                            out=kp0[:, 0:512], in0=pk0[:], scalar1=kb_sb
                        )
                        for k2 in range(2):
                            pv = psm.tile([128, 512], f32, tag="misc", name="pv0")
                            for u in range(2):
                                k = k2 * 2 + u
                                pvs = pv[:, u * 256 : (u + 1) * 256]
                                nc.tensor.matmul(pvs, xT[:, k, 0], wvp[0], start=True, stop=False)
                                nc.tensor.matmul(pvs, xT[:, k, 1], wvp[1], start=False, stop=True)
                                nc.vector.tensor_copy(out=vp0[:, k, :CX], in_=pvs)
                        qp, kp, vp = state["qp"], state["kp"], state["vp"]
                        produce, consume, consume_last = make_jb(b, qp, kp, vp, 0)
                        for q in range(1, 4):
                            transpose_quad(q, xtbs[q])
                            produce(q - 1)
                            proj_quad(b, q)
                            if q >= 2:
                                consume(q - 2)
                        produce(3)
                        consume(2)
                        consume_last()
                        jb_start = 1
                    else:
                        qp, kp, vp = state["qp"], state["kp"], state["vp"]
                        jb_start = 0

                    for jb in range(jb_start, NJB):
                        # prefetch x(b+1) at the batch head; transposes one jb
                        # later so their cast waits never stall the SP queue
                        if not last and jb == jb_start:
                            alloc_batch(b + 1)
                            xtbs = [load_quad(b + 1, q) for q in range(4)]
                        if not last and jb == jb_start + 1:
                            for q in range(4):
                                transpose_quad(q, xtbs[q])
                        attend_jb(b, qp, kp, vp, jb, tail=last and jb >= NJB - 2)
                        if not last:
                            pj = jb - 4
                            if 0 <= pj < 4:
                                proj_quad(b + 1, pj)

    nc.compile()
    return nc


def kernel(x, W_qkv, b_qkv, W_out, b_out):
    global _COMPILED
    from concourse import bass_utils

    x = np.ascontiguousarray(np.asarray(x, dtype=np.float32).reshape(B, S, CX))
    W_qkv = np.asarray(W_qkv, dtype=np.float32)
    b_qkv = np.asarray(b_qkv, dtype=np.float32)
    W_out = np.asarray(W_out, dtype=np.float32)
    b_out = np.asarray(b_out, dtype=np.float32)

    if _COMPILED is None:
        _COMPILED = _build_program()
    nc = _COMPILED

    in_maps = []
    vbp_sum = np.zeros((C,), np.float64)
    for h in range(NCORES):
        w, vbp = _head_weights(h, W_qkv, b_qkv, W_out)
        vbp_sum += vbp.astype(np.float64)
        in_maps.append({"x": x, **w})

    try:
        trace = bool(int(os.environ.get("BASS_PROFILE", "0")))
    except ValueError:
        trace = False
    try:
        res = bass_utils.run_bass_kernel_spmd(
            nc, in_maps, core_ids=list(range(NCORES)), trace=trace
        )
    except Exception:
        # transient NRT_EXEC_UNIT_UNRECOVERABLE observed on the tunneled
        # device; a fresh attempt recovers
        import time as _time

        _time.sleep(2.0)
        res = bass_utils.run_bass_kernel_spmd(
            nc, in_maps, core_ids=list(range(NCORES)), trace=trace
        )
    if trace:
        kernel.last_exec_time_ns = res.exec_time_ns
        kernel.last_results = res

    y = np.zeros((B, S, C, X), dtype=np.float64)
    for h in range(NCORES):
        y += res.results[h]["y"].astype(np.float64).reshape(B, S, C, X)
    # attention-invariant per-head v-bias contribution + output bias,
    # both on the scalar blade only
    y[:, :, :, 0] += (vbp_sum + b_out.astype(np.float64))[None, None, :]
    return y.astype(np.float32)


# revision 25
# speedup vs baseline: 1.1328x; 1.1328x over previous
"""EquiMultiHeadAttention on 8 Trainium2 NeuronCores.

Sharding: one attention head per core (H=8, n_cores=8). Each core computes,
for all 4 batches, its head's q/k/v projections, the full SxS attention, and
that head's contribution to the output projection. The host sums the 8
partial outputs and adds the output bias (scalar blade only) plus the
attention-invariant v-bias contribution (softmax weights sum to exactly 1,
so the per-head v bias commutes with attention and is applied on the host).

Math folded into per-head host-precomputed weights:
  - q is packed to the 8 surviving mv components of the PGA inner product,
    pre-scaled by 1/sqrt(32); k packed identically -> the score matmul is a
    plain K=128 contraction.
  - The output projection (W_out columns of this head) is applied to v
    *before* attention, so the attention's second matmul directly produces
    this head's output contribution. An extra all-ones column on v yields
    the softmax denominator in the same matmul.

Whole pipeline runs in bf16 (measured end-to-end rel err ~8e-3):
  - x is cast to bf16 on the Pool engine and transposed to [(c,x), s] layout
    by the DMA xbar transpose (one instruction per 512-token quad; no PE
    transposes, no PSUM->SBUF transpose copies).
  - All matmuls are bf16 (1 cycle/row on the PE).
  - Scores are exp'd by the scalar engine in 1024-element groups
    (4 i-blocks x 256 j) straight out of PSUM into bf16 SBUF tiles.
  - x for batch b+1 is loaded/cast/transposed during the first half of
    batch b's attention and projected during the second half.
"""

import sys
import os

sys.path.insert(0, "/opt/trn_rl_repo")

import numpy as np
import ml_dtypes

B, S, C, X = 4, 2048, 16, 16
H = 8
CX = C * X  # 256
SURV = [0, 2, 3, 4, 8, 9, 10, 14]  # mv components surviving <q, ~k>
SCALE = 1.0 / np.sqrt(32.0)
NCORES = 8
SB, JB, IB = 128, 256, 128  # s-tile, j-block, i-block sizes
NST, NJB, NIB = S // SB, S // JB, S // IB  # 16, 8, 16
GRP = 4  # i-blocks per exp group
NG = NIB // GRP  # 4 groups
NV = CX + 1  # 257: v columns + denominator ones column
NWALL = 1024  # packed weight image columns (wq 256 | wk 256 | wvp 512)

_COMPILED = None


def _head_weights(h, W_qkv, b_qkv, W_out):
    """Per-head block-diagonal weight construction (bf16 device weights)."""
    f32 = np.float32
    bf16 = ml_dtypes.bfloat16
    # row h*48 + c'*3 + p  (p: 0=q, 1=k, 2=v)
    Wh = W_qkv[h * 48 : (h + 1) * 48].reshape(C, 3, C)  # [c', p, c]
    bh = b_qkv[h * 48 : (h + 1) * 48].reshape(C, 3)  # [c', p]
    Wq, Wk, Wv = Wh[:, 0], Wh[:, 1], Wh[:, 2]  # each [c', c]
    qb, kb, vb = bh[:, 0], bh[:, 1], bh[:, 2]
    Wout_h = W_out[:, np.arange(C) * H + h]  # [o, c']
    Wvp = Wout_h @ Wv  # [o, c]
    vbp = Wout_h @ vb  # [o] -> host-side add

    # x_T row layout within half: r = (c - half*8)*16 + xi
    # packed q/k column layout: d = c'*8 + si  (si indexes SURV)
    Wq_bd = np.zeros((2, 128, 128), f32)
    Wk_bd = np.zeros((2, 128, 128), f32)
    Wvp_bd = np.zeros((2, 128, CX), f32)
    for half in range(2):
        for cl in range(8):
            c = half * 8 + cl
            for si, xs in enumerate(SURV):
                r = cl * 16 + xs
                Wq_bd[half, r, np.arange(C) * 8 + si] = SCALE * Wq[:, c]
                Wk_bd[half, r, np.arange(C) * 8 + si] = Wk[:, c]
            for xi in range(16):
                r = cl * 16 + xi
                Wvp_bd[half, r, np.arange(C) * 16 + xi] = Wvp[:, c]
    qb_col = np.zeros((128, 1), f32)
    kb_col = np.zeros((128, 1), f32)
    qb_col[np.arange(C) * 8, 0] = SCALE * qb  # si=0 <-> x component 0
    kb_col[np.arange(C) * 8, 0] = kb
    # single packed bf16 weight image: [wq(2x128) | wk(2x128) | wvp(2x256)]
    # plus a tiny f32 bias pair -> two DMAs instead of eight
    W_all = np.zeros((128, NWALL), f32)
    W_all[:, 0:256] = Wq_bd.transpose(1, 0, 2).reshape(128, 256)
    W_all[:, 256:512] = Wk_bd.transpose(1, 0, 2).reshape(128, 256)
    W_all[:, 512:1024] = Wvp_bd.transpose(1, 0, 2).reshape(128, 512)
    qkb = np.concatenate([qb_col, kb_col], axis=1)  # [128, 2] f32
    return {"W_all": W_all.astype(bf16), "qkb": qkb}, vbp


def _build_program():
    import concourse.bass as bass
    import concourse.mybir as mybir
    import concourse.tile as tile
    from concourse import bacc

    f32 = mybir.dt.float32
    bf16 = mybir.dt.bfloat16
    Exp = mybir.ActivationFunctionType.Exp

    nc = bacc.Bacc("TRN2", target_bir_lowering=False, debug=False)

    x_d = nc.dram_tensor("x", [B, S, CX], f32, kind="ExternalInput").ap()
    wall_d = nc.dram_tensor("W_all", [128, NWALL], bf16, kind="ExternalInput").ap()
    qkb_d = nc.dram_tensor("qkb", [128, 2], f32, kind="ExternalInput").ap()
    y_d = nc.dram_tensor("y", [B, S, CX], f32, kind="ExternalOutput").ap()

    with tile.TileContext(nc) as tc:
        with (
            tc.tile_pool(name="const", bufs=1) as const,
            tc.tile_pool(name="xin", bufs=8) as xin,
            tc.tile_pool(name="xtb", bufs=8) as xtbp,
            tc.tile_pool(name="xT", bufs=2) as xTp,
            tc.tile_pool(name="qk", bufs=2) as qkp,
            tc.tile_pool(name="vp", bufs=2) as vpp,
            tc.tile_pool(name="es", bufs=3) as esp,
            tc.tile_pool(name="yo", bufs=3) as yop,
            tc.tile_pool(name="psm", bufs=2, space="PSUM") as psm,
            tc.tile_pool(name="pss", bufs=2, space="PSUM") as pssp,
            tc.tile_pool(name="psy", bufs=1, space="PSUM") as psyp,
        ):
            state = {}

            def load_consts():
                wall = const.tile([128, NWALL], bf16, tag="wall", name="wall")
                nc.scalar.dma_start(out=wall[:], in_=wall_d[:])
                state["wq"] = [wall[:, h * 128 : (h + 1) * 128] for h in range(2)]
                state["wk"] = [wall[:, 256 + h * 128 : 256 + (h + 1) * 128] for h in range(2)]
                state["wvp"] = [wall[:, 512 + h * CX : 512 + (h + 1) * CX] for h in range(2)]
                qkb = const.tile([128, 2], f32, tag="qkb", name="qkb")
                nc.scalar.dma_start(out=qkb[:], in_=qkb_d[:])
                state["qb_sb"] = qkb[:, 0:1]
                state["kb_sb"] = qkb[:, 1:2]

            try:
                n_rep = int(os.environ.get("BASS_REPEAT", "1"))
            except ValueError:
                n_rep = 1

            def alloc_batch(b):
                # xT layout: [c-in-half, s-tile, half, s-within-tile]
                state["xT"] = xTp.tile([128, NST, 2, SB], bf16, tag="xT", name=f"xT{b}")
                state["qp"] = qkp.tile([128, S], bf16, tag="qp", name=f"qp{b}")
                state["kp"] = qkp.tile([128, S], bf16, tag="kp", name=f"kp{b}")
                vp = vpp.tile([128, NST, NV], bf16, tag="vp", name=f"vp{b}")
                # denominator ones column (the v bias itself is added on host)
                nc.gpsimd.memset(vp[:, :, CX : CX + 1], 1.0)
                state["vp"] = vp

            def load_quad(b, q):
                """DMA a 512-token quad of x and cast it to bf16 (Pool)."""
                xt = xin.tile([128, 4, CX], f32, tag="x", name="xt")
                src_ap = x_d[b, q * 512 : (q + 1) * 512, :].rearrange(
                    "(k p) c -> p k c", k=4, p=SB
                )
                nc.sync.dma_start(out=xt[:], in_=src_ap)
                xtb = xtbp.tile([128, 4, CX], bf16, tag="xtb", name="xtb")
                nc.gpsimd.tensor_copy(out=xtb[:], in_=xt[:])
                return xtb

            def transpose_quad(q, xtb):
                # out view [c, (st half), s] merges the st/half dims (contiguous)
                xT = state["xT"]
                dst = xT[:, 4 * q : 4 * q + 4].rearrange("c k h s -> c (k h) s")
                nc.sync.dma_start_transpose(
                    out=dst, in_=xtb[:].rearrange("p k c -> p (k c)")
                )

            def proj_quad(b, q):
                """Project one 512-token quad into qp/kp (bf16) and vp (bf16)."""
                wq, wk, wvp = state["wq"], state["wk"], state["wvp"]
                qb_sb, kb_sb = state["qb_sb"], state["kb_sb"]
                xT, qp, kp, vp = state["xT"], state["qp"], state["kp"], state["vp"]
                sl = slice(q * 512, (q + 1) * 512)
                stq = slice(4 * q, 4 * q + 4)
                pq = psm.tile([128, 512], f32, tag="misc", name="pq")
                nc.tensor.matmul(pq[:], wq[0], xT[:, stq, 0], start=True, stop=False)
                nc.tensor.matmul(pq[:], wq[1], xT[:, stq, 1], start=False, stop=True)
                nc.vector.tensor_scalar_add(out=qp[:, sl], in0=pq[:], scalar1=qb_sb)
                pk = psm.tile([128, 512], f32, tag="misc", name="pk")
                nc.tensor.matmul(pk[:], wk[0], xT[:, stq, 0], start=True, stop=False)
                nc.tensor.matmul(pk[:], wk[1], xT[:, stq, 1], start=False, stop=True)
                nc.vector.tensor_scalar_add(out=kp[:, sl], in0=pk[:], scalar1=kb_sb)
                for st2 in range(q * 2, q * 2 + 2):
                    pv = psm.tile([128, 512], f32, tag="misc", name="pv")
                    for u in range(2):
                        st = st2 * 2 + u
                        pvs = pv[:, u * 256 : (u + 1) * 256]
                        nc.tensor.matmul(pvs, xT[:, st, 0], wvp[0], start=True, stop=False)
                        nc.tensor.matmul(pvs, xT[:, st, 1], wvp[1], start=False, stop=True)
                        nc.vector.tensor_copy(out=vp[:, st, :CX], in_=pvs)

            def make_jb(b, qp, kp, vp, jb, tail=False):
                jsl = slice(jb * JB, (jb + 1) * JB)
                yps = [
                    psyp.tile([128, NV], f32, tag=f"yps{js}", name=f"yps{js}")
                    for js in range(2)
                ]
                es_q = {}

                def produce(g):
                    pss = pssp.tile([128, GRP, JB], f32, tag="ps_s", name="pss")
                    for gi in range(GRP):
                        ib = g * GRP + gi
                        isl = slice(ib * IB, (ib + 1) * IB)
                        nc.tensor.matmul(
                            pss[:, gi], kp[:, isl], qp[:, jsl], start=True, stop=True
                        )
                    es = esp.tile([128, GRP, JB], bf16, tag="es", name="es")
                    nc.scalar.activation(es[:], pss[:], Exp)
                    es_q[g] = es

                def consume(g):
                    # the last group runs js-major with an early PSUM release:
                    # each yps bank is copied out right after its final matmul
                    # so the next jb's accumulation can begin immediately
                    es = es_q.pop(g)
                    if g < NG - 1:
                        for gi in range(GRP):
                            ib = g * GRP + gi
                            for js in range(2):
                                nc.tensor.matmul(
                                    yps[js][:],
                                    es[:, gi, js * 128 : (js + 1) * 128],
                                    vp[:, ib],
                                    start=(ib == 0),
                                    stop=(ib == NIB - 1),
                                )
                        return
                    ysb = yop.tile([128, 2, CX], f32, tag="ysb")
                    dst_ap = y_d[b, jb * JB : (jb + 1) * JB, :].rearrange(
                        "(k p) c -> p k c", k=2, p=SB
                    )
                    for js in range(2):
                        for gi in range(GRP):
                            ib = g * GRP + gi
                            nc.tensor.matmul(
                                yps[js][:],
                                es[:, gi, js * 128 : (js + 1) * 128],
                                vp[:, ib],
                                start=(ib == 0),
                                stop=(ib == NIB - 1),
                            )
                        yz = yop.tile([128, NV], f32, tag="yz")
                        nc.vector.tensor_copy(out=yz[:], in_=yps[js][:])
                        rec = yop.tile([128, 1], f32, tag="rec")
                        nc.vector.reciprocal(rec[:], yz[:, CX : CX + 1])
                        nc.vector.tensor_scalar_mul(
                            out=ysb[:, js], in0=yz[:, :CX], scalar1=rec[:]
                        )
                        eng = nc.sync if tail else nc.gpsimd
                        eng.dma_start(out=dst_ap[:, js], in_=ysb[:, js])

                return produce, consume

            for rep in range(n_rep):
                for b in range(B):
                    first = rep == 0 and b == 0
                    last = rep == n_rep - 1 and b == B - 1
                    if first:
                        # cold start: consts first, then quad 0 streamed at
                        # 128-token chunk granularity so the first projection
                        # matmuls start as early as possible
                        alloc_batch(b)
                        xT = state["xT"]
                        src0 = x_d[b, 0:512, :].rearrange("(k p) c -> p k c", k=4, p=SB)
                        xt0 = xin.tile([128, 4, CX], f32, tag="x", name="xt0")
                        xtb0 = xtbp.tile([128, 4, CX], bf16, tag="xtb", name="xtb0")
                        for k in range(2):
                            nc.sync.dma_start(out=xt0[:, k], in_=src0[:, k])
                        load_consts()
                        for k in range(2, 4):
                            nc.sync.dma_start(out=xt0[:, k], in_=src0[:, k])
                        for k in range(4):
                            nc.gpsimd.tensor_copy(out=xtb0[:, k], in_=xt0[:, k])
                        # all x DMAs dispatched before any (cast-waiting)
                        # transpose: DMA waits block the whole SP queue
                        xtbs = [None] + [load_quad(b, q) for q in range(1, 4)]
                        for k in range(4):
                            nc.sync.dma_start_transpose(out=xT[:, k], in_=xtb0[:, k])
                        # chunked projections of quad 0 into shared pss tiles
                        wq, wk, wvp = state["wq"], state["wk"], state["wvp"]
                        qb_sb, kb_sb = state["qb_sb"], state["kb_sb"]
                        qp0, kp0, vp0 = state["qp"], state["kp"], state["vp"]
                        pq0 = psm.tile([128, 512], f32, tag="misc", name="pq0")
                        pk0 = psm.tile([128, 512], f32, tag="misc", name="pk0")
                        for k in range(4):
                            oq = pq0[:, k * 128 : (k + 1) * 128]
                            ok = pk0[:, k * 128 : (k + 1) * 128]
                            nc.tensor.matmul(oq, wq[0], xT[:, k, 0], start=True, stop=False)
                            nc.tensor.matmul(oq, wq[1], xT[:, k, 1], start=False, stop=True)
                            nc.tensor.matmul(ok, wk[0], xT[:, k, 0], start=True, stop=False)
                            nc.tensor.matmul(ok, wk[1], xT[:, k, 1], start=False, stop=True)
                        nc.vector.tensor_scalar_add(
                            out=qp0[:, 0:512], in0=pq0[:], scalar1=qb_sb
                        )
                        nc.vector.tensor_scalar_add(
                            out=kp0[:, 0:512], in0=pk0[:], scalar1=kb_sb
                        )
                        for k2 in range(2):
                            pv = psm.tile([128, 512], f32, tag="misc", name="pv0")
                            for u in range(2):
                                k = k2 * 2 + u
                                pvs = pv[:, u * 256 : (u + 1) * 256]
                                nc.tensor.matmul(pvs, xT[:, k, 0], wvp[0], start=True, stop=False)
                                nc.tensor.matmul(pvs, xT[:, k, 1], wvp[1], start=False, stop=True)
                                nc.vector.tensor_copy(out=vp0[:, k, :CX], in_=pvs)
                        qp, kp, vp = state["qp"], state["kp"], state["vp"]
                        produce, consume = make_jb(b, qp, kp, vp, 0)
                        for q in range(1, 4):
                            transpose_quad(q, xtbs[q])
                            produce(q - 1)
                            proj_quad(b, q)
                            if q >= 2:
                                consume(q - 2)
                        produce(3)
                        consume(2)
                        consume(3)
                        jb_start = 1
                    else:
                        qp, kp, vp = state["qp"], state["kp"], state["vp"]
                        jb_start = 0

                    # flat lag-2 software pipeline over (jb, g) groups:
                    # consume trails produce by 2 groups so neither the pss
                    # ring nor the yps release is ever on the critical path
                    stream = [
                        (jb, g) for jb in range(jb_start, NJB) for g in range(NG)
                    ]
                    jbs = {}
                    pending = []

                    def hooks(jb):
                        if last:
                            return
                        if jb == jb_start:
                            alloc_batch(b + 1)
                            state["xtbs"] = [load_quad(b + 1, q) for q in range(4)]
                        elif jb == jb_start + 1:
                            for q in range(4):
                                transpose_quad(q, state["xtbs"][q])
                        elif jb >= 4:
                            proj_quad(b + 1, jb - 4)

                    for idx, (jb, g) in enumerate(stream):
                        if g == 0:
                            hooks(jb)
                            jbs[jb] = make_jb(
                                b, qp, kp, vp, jb, tail=last and jb >= NJB - 2
                            )
                        jbs[jb][0](g)
                        pending.append((jb, g))
                        if idx >= 2:
                            cjb, cg = pending.pop(0)
                            jbs[cjb][1](cg)
                    for cjb, cg in pending:
                        jbs[cjb][1](cg)

    nc.compile()
    return nc


def kernel(x, W_qkv, b_qkv, W_out, b_out):
    global _COMPILED
    from concourse import bass_utils

    x = np.ascontiguousarray(np.asarray(x, dtype=np.float32).reshape(B, S, CX))
    W_qkv = np.asarray(W_qkv, dtype=np.float32)
    b_qkv = np.asarray(b_qkv, dtype=np.float32)
    W_out = np.asarray(W_out, dtype=np.float32)
    b_out = np.asarray(b_out, dtype=np.float32)

    if _COMPILED is None:
        _COMPILED = _build_program()
    nc = _COMPILED

    in_maps = []
    vbp_sum = np.zeros((C,), np.float64)
    for h in range(NCORES):
        w, vbp = _head_weights(h, W_qkv, b_qkv, W_out)
        vbp_sum += vbp.astype(np.float64)
        in_maps.append({"x": x, **w})

    try:
        trace = bool(int(os.environ.get("BASS_PROFILE", "0")))
    except ValueError:
        trace = False
    try:
        res = bass_utils.run_bass_kernel_spmd(
            nc, in_maps, core_ids=list(range(NCORES)), trace=trace
        )
    except Exception:
        # transient NRT_EXEC_UNIT_UNRECOVERABLE observed on the tunneled
        # device; a fresh attempt recovers
        import time as _time

        _time.sleep(2.0)
        res = bass_utils.run_bass_kernel_spmd(
            nc, in_maps, core_ids=list(range(NCORES)), trace=trace
        )
    if trace:
        kernel.last_exec_time_ns = res.exec_time_ns
        kernel.last_results = res

    y = np.zeros((B, S, C, X), dtype=np.float64)
    for h in range(NCORES):
        y += res.results[h]["y"].astype(np.float64).reshape(B, S, C, X)
    # attention-invariant per-head v-bias contribution + output bias,
    # both on the scalar blade only
    y[:, :, :, 0] += (vbp_sum + b_out.astype(np.float64))[None, None, :]
    return y.astype(np.float32)


# revision 30
# speedup vs baseline: 1.1602x; 1.0242x over previous
"""EquiMultiHeadAttention on 8 Trainium2 NeuronCores.

Sharding: one attention head per core (H=8, n_cores=8). Each core computes,
for all 4 batches, its head's q/k/v projections, the full SxS attention, and
that head's contribution to the output projection. The host sums the 8
partial outputs and adds the output bias (scalar blade only) plus the
attention-invariant v-bias contribution (softmax weights sum to exactly 1,
so the per-head v bias commutes with attention and is applied on the host).

Math folded into per-head host-precomputed weights:
  - q is packed to the 8 surviving mv components of the PGA inner product,
    pre-scaled by 1/sqrt(32); k packed identically -> the score matmul is a
    plain K=128 contraction.
  - The output projection (W_out columns of this head) is applied to v
    *before* attention, so the attention's second matmul directly produces
    this head's output contribution. An extra all-ones column on v yields
    the softmax denominator in the same matmul.

Whole pipeline runs in bf16 (measured end-to-end rel err ~8e-3):
  - x is cast to bf16 on the Pool engine and transposed to [(c,x), s] layout
    by the DMA xbar transpose (one instruction per 512-token quad; no PE
    transposes, no PSUM->SBUF transpose copies).
  - All matmuls are bf16 (1 cycle/row on the PE).
  - Scores are exp'd by the scalar engine in 1024-element groups
    (4 i-blocks x 256 j) straight out of PSUM into bf16 SBUF tiles.
  - x for batch b+1 is loaded/cast/transposed during the first half of
    batch b's attention and projected during the second half.
"""

import sys
import os

sys.path.insert(0, "/opt/trn_rl_repo")

import numpy as np
import ml_dtypes

B, S, C, X = 4, 2048, 16, 16
H = 8
CX = C * X  # 256
SURV = [0, 2, 3, 4, 8, 9, 10, 14]  # mv components surviving <q, ~k>
SCALE = 1.0 / np.sqrt(32.0)
NCORES = 8
SB, JB, IB = 128, 256, 128  # s-tile, j-block, i-block sizes
NST, NJB, NIB = S // SB, S // JB, S // IB  # 16, 8, 16
GRP = 4  # i-blocks per exp group
NG = NIB // GRP  # 4 groups
NV = CX + 1  # 257: v columns + denominator ones column
NWALL = 1024  # packed weight image columns (wq 256 | wk 256 | wvp 512)

_COMPILED = None


def _head_weights(h, W_qkv, b_qkv, W_out):
    """Per-head block-diagonal weight construction (bf16 device weights)."""
    f32 = np.float32
    bf16 = ml_dtypes.bfloat16
    # row h*48 + c'*3 + p  (p: 0=q, 1=k, 2=v)
    Wh = W_qkv[h * 48 : (h + 1) * 48].reshape(C, 3, C)  # [c', p, c]
    bh = b_qkv[h * 48 : (h + 1) * 48].reshape(C, 3)  # [c', p]
    Wq, Wk, Wv = Wh[:, 0], Wh[:, 1], Wh[:, 2]  # each [c', c]
    qb, kb, vb = bh[:, 0], bh[:, 1], bh[:, 2]
    Wout_h = W_out[:, np.arange(C) * H + h]  # [o, c']
    Wvp = Wout_h @ Wv  # [o, c]
    vbp = Wout_h @ vb  # [o] -> host-side add

    # x_T row layout within half: r = (c - half*8)*16 + xi
    # packed q/k column layout: d = c'*8 + si  (si indexes SURV)
    Wq_bd = np.zeros((2, 128, 128), f32)
    Wk_bd = np.zeros((2, 128, 128), f32)
    Wvp_bd = np.zeros((2, 128, CX), f32)
    for half in range(2):
        for cl in range(8):
            c = half * 8 + cl
            for si, xs in enumerate(SURV):
                r = cl * 16 + xs
                Wq_bd[half, r, np.arange(C) * 8 + si] = SCALE * Wq[:, c]
                Wk_bd[half, r, np.arange(C) * 8 + si] = Wk[:, c]
            for xi in range(16):
                r = cl * 16 + xi
                Wvp_bd[half, r, np.arange(C) * 16 + xi] = Wvp[:, c]
    qb_col = np.zeros((128, 1), f32)
    kb_col = np.zeros((128, 1), f32)
    qb_col[np.arange(C) * 8, 0] = SCALE * qb  # si=0 <-> x component 0
    kb_col[np.arange(C) * 8, 0] = kb
    # single packed bf16 weight image: [wq(2x128) | wk(2x128) | wvp(2x256)]
    # plus a tiny f32 bias pair -> two DMAs instead of eight
    W_all = np.zeros((128, NWALL), f32)
    W_all[:, 0:256] = Wq_bd.transpose(1, 0, 2).reshape(128, 256)
    W_all[:, 256:512] = Wk_bd.transpose(1, 0, 2).reshape(128, 256)
    W_all[:, 512:1024] = Wvp_bd.transpose(1, 0, 2).reshape(128, 512)
    qkb = np.concatenate([qb_col, kb_col], axis=1)  # [128, 2] f32
    return {"W_all": W_all.astype(bf16), "qkb": qkb}, vbp


def _build_program():
    import concourse.bass as bass
    import concourse.mybir as mybir
    import concourse.tile as tile
    from concourse import bacc

    f32 = mybir.dt.float32
    bf16 = mybir.dt.bfloat16
    Exp = mybir.ActivationFunctionType.Exp

    nc = bacc.Bacc("TRN2", target_bir_lowering=False, debug=False)

    x_d = nc.dram_tensor("x", [B, S, CX], bf16, kind="ExternalInput").ap()
    wall_d = nc.dram_tensor("W_all", [128, NWALL], bf16, kind="ExternalInput").ap()
    qkb_d = nc.dram_tensor("qkb", [128, 2], f32, kind="ExternalInput").ap()
    y_d = nc.dram_tensor("y", [B, S, CX], f32, kind="ExternalOutput").ap()

    with tile.TileContext(nc) as tc:
        with (
            tc.tile_pool(name="const", bufs=1) as const,
            tc.tile_pool(name="xin", bufs=8) as xin,
            tc.tile_pool(name="xT", bufs=2) as xTp,
            tc.tile_pool(name="qk", bufs=2) as qkp,
            tc.tile_pool(name="vp", bufs=2) as vpp,
            tc.tile_pool(name="es", bufs=3) as esp,
            tc.tile_pool(name="yo", bufs=3) as yop,
            tc.tile_pool(name="psm", bufs=2, space="PSUM") as psm,
            tc.tile_pool(name="pss", bufs=2, space="PSUM") as pssp,
            tc.tile_pool(name="psy", bufs=1, space="PSUM") as psyp,
        ):
            state = {}

            def load_consts():
                wall = const.tile([128, NWALL], bf16, tag="wall", name="wall")
                nc.scalar.dma_start(out=wall[:], in_=wall_d[:])
                state["wq"] = [wall[:, h * 128 : (h + 1) * 128] for h in range(2)]
                state["wk"] = [wall[:, 256 + h * 128 : 256 + (h + 1) * 128] for h in range(2)]
                state["wvp"] = [wall[:, 512 + h * CX : 512 + (h + 1) * CX] for h in range(2)]
                qkb = const.tile([128, 2], f32, tag="qkb", name="qkb")
                nc.scalar.dma_start(out=qkb[:], in_=qkb_d[:])
                state["qb_sb"] = qkb[:, 0:1]
                state["kb_sb"] = qkb[:, 1:2]

            try:
                n_rep = int(os.environ.get("BASS_REPEAT", "1"))
            except ValueError:
                n_rep = 1

            def alloc_batch(b):
                # xT layout: [c-in-half, s-tile, half, s-within-tile]
                state["xT"] = xTp.tile([128, NST, 2, SB], bf16, tag="xT", name=f"xT{b}")
                state["qp"] = qkp.tile([128, S], bf16, tag="qp", name=f"qp{b}")
                state["kp"] = qkp.tile([128, S], bf16, tag="kp", name=f"kp{b}")
                vp = vpp.tile([128, NST, NV], bf16, tag="vp", name=f"vp{b}")
                # denominator ones column (the v bias itself is added on host)
                nc.gpsimd.memset(vp[:, :, CX : CX + 1], 1.0)
                state["vp"] = vp

            def load_quad(b, q):
                """DMA a 512-token quad of x (already bf16 from the host)."""
                xt = xin.tile([128, 4, CX], bf16, tag="x", name="xt")
                src_ap = x_d[b, q * 512 : (q + 1) * 512, :].rearrange(
                    "(k p) c -> p k c", k=4, p=SB
                )
                nc.sync.dma_start(out=xt[:], in_=src_ap)
                return xt

            def transpose_quad(q, xtb):
                # out view [c, (st half), s] merges the st/half dims (contiguous)
                xT = state["xT"]
                dst = xT[:, 4 * q : 4 * q + 4].rearrange("c k h s -> c (k h) s")
                nc.sync.dma_start_transpose(
                    out=dst, in_=xtb[:].rearrange("p k c -> p (k c)")
                )

            def proj_quad(b, q):
                """Project one 512-token quad into qp/kp (bf16) and vp (bf16)."""
                wq, wk, wvp = state["wq"], state["wk"], state["wvp"]
                qb_sb, kb_sb = state["qb_sb"], state["kb_sb"]
                xT, qp, kp, vp = state["xT"], state["qp"], state["kp"], state["vp"]
                sl = slice(q * 512, (q + 1) * 512)
                stq = slice(4 * q, 4 * q + 4)
                pq = psm.tile([128, 512], f32, tag="misc", name="pq")
                nc.tensor.matmul(pq[:], wq[0], xT[:, stq, 0], start=True, stop=False)
                nc.tensor.matmul(pq[:], wq[1], xT[:, stq, 1], start=False, stop=True)
                nc.vector.tensor_scalar_add(out=qp[:, sl], in0=pq[:], scalar1=qb_sb)
                pk = psm.tile([128, 512], f32, tag="misc", name="pk")
                nc.tensor.matmul(pk[:], wk[0], xT[:, stq, 0], start=True, stop=False)
                nc.tensor.matmul(pk[:], wk[1], xT[:, stq, 1], start=False, stop=True)
                nc.vector.tensor_scalar_add(out=kp[:, sl], in0=pk[:], scalar1=kb_sb)
                for st2 in range(q * 2, q * 2 + 2):
                    pv = psm.tile([128, 512], f32, tag="misc", name="pv")
                    for u in range(2):
                        st = st2 * 2 + u
                        pvs = pv[:, u * 256 : (u + 1) * 256]
                        nc.tensor.matmul(pvs, xT[:, st, 0], wvp[0], start=True, stop=False)
                        nc.tensor.matmul(pvs, xT[:, st, 1], wvp[1], start=False, stop=True)
                        nc.vector.tensor_copy(out=vp[:, st, :CX], in_=pvs)

            def make_jb(b, qp, kp, vp, jb, tail=False):
                jsl = slice(jb * JB, (jb + 1) * JB)
                yps = [
                    psyp.tile([128, NV], f32, tag=f"yps{js}", name=f"yps{js}")
                    for js in range(2)
                ]
                es_q = {}

                def produce(g):
                    pss = pssp.tile([128, GRP, JB], f32, tag="ps_s", name="pss")
                    for gi in range(GRP):
                        ib = g * GRP + gi
                        isl = slice(ib * IB, (ib + 1) * IB)
                        nc.tensor.matmul(
                            pss[:, gi], kp[:, isl], qp[:, jsl], start=True, stop=True
                        )
                    es = esp.tile([128, GRP, JB], bf16, tag="es", name="es")
                    nc.scalar.activation(es[:], pss[:], Exp)
                    es_q[g] = es

                def consume(g):
                    # the last group runs js-major with an early PSUM release:
                    # each yps bank is copied out right after its final matmul
                    # so the next jb's accumulation can begin immediately
                    es = es_q.pop(g)
                    if g < NG - 1:
                        for gi in range(GRP):
                            ib = g * GRP + gi
                            for js in range(2):
                                nc.tensor.matmul(
                                    yps[js][:],
                                    es[:, gi, js * 128 : (js + 1) * 128],
                                    vp[:, ib],
                                    start=(ib == 0),
                                    stop=(ib == NIB - 1),
                                )
                        return
                    ysb = yop.tile([128, 2, CX], f32, tag="ysb")
                    dst_ap = y_d[b, jb * JB : (jb + 1) * JB, :].rearrange(
                        "(k p) c -> p k c", k=2, p=SB
                    )
                    for js in range(2):
                        for gi in range(GRP):
                            ib = g * GRP + gi
                            nc.tensor.matmul(
                                yps[js][:],
                                es[:, gi, js * 128 : (js + 1) * 128],
                                vp[:, ib],
                                start=(ib == 0),
                                stop=(ib == NIB - 1),
                            )
                        if tail:
                            src_n = yps[js]
                        else:
                            src_n = yop.tile([128, NV], f32, tag="yz")
                            nc.vector.tensor_copy(out=src_n[:], in_=yps[js][:])
                        rec = yop.tile([128, 1], f32, tag="rec")
                        nc.vector.reciprocal(rec[:], src_n[:, CX : CX + 1])
                        nc.vector.tensor_scalar_mul(
                            out=ysb[:, js], in0=src_n[:, :CX], scalar1=rec[:]
                        )
                        eng = nc.sync if tail else nc.gpsimd
                        eng.dma_start(out=dst_ap[:, js], in_=ysb[:, js])

                return produce, consume

            for rep in range(n_rep):
                for b in range(B):
                    first = rep == 0 and b == 0
                    last = rep == n_rep - 1 and b == B - 1
                    if first:
                        # cold start: x quad DMAs + consts first (HWDGE
                        # dispatch is the serial resource), casts split
                        # across DVE/Pool, then transposes, then the jb0
                        # attention interleaved with per-quad projections
                        alloc_batch(b)
                        xT = state["xT"]
                        xtbs = []
                        for q in range(4):
                            xt = xin.tile([128, 4, CX], bf16, tag="x", name=f"xt0{q}")
                            src_ap = x_d[b, q * 512 : (q + 1) * 512, :].rearrange(
                                "(k p) c -> p k c", k=4, p=SB
                            )
                            nc.sync.dma_start(out=xt[:], in_=src_ap)
                            if q == 0:
                                load_consts()
                            xtbs.append(xt)
                        for q in range(4):
                            transpose_quad(q, xtbs[q])
                        proj_quad(b, 0)
                        qp, kp, vp = state["qp"], state["kp"], state["vp"]
                        produce, consume = make_jb(b, qp, kp, vp, 0)
                        for q in range(1, 4):
                            produce(q - 1)
                            proj_quad(b, q)
                            if q >= 2:
                                consume(q - 2)
                        produce(3)
                        consume(2)
                        consume(3)
                        jb_start = 1
                    else:
                        qp, kp, vp = state["qp"], state["kp"], state["vp"]
                        jb_start = 0

                    # flat lag-2 software pipeline over (jb, g) groups:
                    # consume trails produce by 2 groups so neither the pss
                    # ring nor the yps release is ever on the critical path
                    stream = [
                        (jb, g) for jb in range(jb_start, NJB) for g in range(NG)
                    ]
                    jbs = {}
                    pending = []

                    def hooks(jb):
                        if last:
                            return
                        if jb == jb_start:
                            alloc_batch(b + 1)
                            xts = [load_quad(b + 1, q) for q in range(4)]
                            for q in range(4):
                                transpose_quad(q, xts[q])
                        elif jb >= 4:
                            proj_quad(b + 1, jb - 4)

                    for idx, (jb, g) in enumerate(stream):
                        if g == 0:
                            hooks(jb)
                            jbs[jb] = make_jb(
                                b, qp, kp, vp, jb, tail=last and jb >= NJB - 2
                            )
                        jbs[jb][0](g)
                        pending.append((jb, g))
                        if idx >= 2:
                            cjb, cg = pending.pop(0)
                            jbs[cjb][1](cg)
                    for cjb, cg in pending:
                        jbs[cjb][1](cg)

    nc.compile()
    return nc


def kernel(x, W_qkv, b_qkv, W_out, b_out):
    global _COMPILED
    from concourse import bass_utils

    x = np.ascontiguousarray(
        np.asarray(x, dtype=np.float32).reshape(B, S, CX).astype(ml_dtypes.bfloat16)
    )
    W_qkv = np.asarray(W_qkv, dtype=np.float32)
    b_qkv = np.asarray(b_qkv, dtype=np.float32)
    W_out = np.asarray(W_out, dtype=np.float32)
    b_out = np.asarray(b_out, dtype=np.float32)

    if _COMPILED is None:
        _COMPILED = _build_program()
    nc = _COMPILED

    in_maps = []
    vbp_sum = np.zeros((C,), np.float64)
    for h in range(NCORES):
        w, vbp = _head_weights(h, W_qkv, b_qkv, W_out)
        vbp_sum += vbp.astype(np.float64)
        in_maps.append({"x": x, **w})

    try:
        trace = bool(int(os.environ.get("BASS_PROFILE", "0")))
    except ValueError:
        trace = False
    try:
        res = bass_utils.run_bass_kernel_spmd(
            nc, in_maps, core_ids=list(range(NCORES)), trace=trace
        )
    except Exception:
        # transient NRT_EXEC_UNIT_UNRECOVERABLE observed on the tunneled
        # device; a fresh attempt recovers
        import time as _time

        _time.sleep(2.0)
        res = bass_utils.run_bass_kernel_spmd(
            nc, in_maps, core_ids=list(range(NCORES)), trace=trace
        )
    if trace:
        kernel.last_exec_time_ns = res.exec_time_ns
        kernel.last_results = res

    y = np.zeros((B, S, C, X), dtype=np.float64)
    for h in range(NCORES):
        y += res.results[h]["y"].astype(np.float64).reshape(B, S, C, X)
    # attention-invariant per-head v-bias contribution + output bias,
    # both on the scalar blade only
    y[:, :, :, 0] += (vbp_sum + b_out.astype(np.float64))[None, None, :]
    return y.astype(np.float32)


# revision 41
# speedup vs baseline: 1.1606x; 1.0004x over previous
"""EquiMultiHeadAttention on 8 Trainium2 NeuronCores.

Sharding: one attention head per core (H=8, n_cores=8). Each core computes,
for all 4 batches, its head's q/k/v projections, the full SxS attention, and
that head's contribution to the output projection. The host sums the 8
partial outputs and adds the output bias (scalar blade only) plus the
attention-invariant v-bias contribution (softmax weights sum to exactly 1,
so the per-head v bias commutes with attention and is applied on the host).

Math folded into per-head host-precomputed weights:
  - q is packed to the 8 surviving mv components of the PGA inner product,
    pre-scaled by 1/sqrt(32); k packed identically -> the score matmul is a
    plain K=128 contraction.
  - The output projection (W_out columns of this head) is applied to v
    *before* attention, so the attention's second matmul directly produces
    this head's output contribution. An extra all-ones column on v yields
    the softmax denominator in the same matmul.

Whole pipeline runs in bf16 (measured end-to-end rel err ~8e-3):
  - x is cast to bf16 on the host and transposed to [(c,x), s] layout by the
    DMA xbar transpose (one instruction per 512-token quad; no PE transposes,
    no PSUM->SBUF transpose copies, no on-device casts).
  - All matmuls are bf16 (1 cycle/row on the PE).
  - Scores are exp'd by the scalar engine in 1024-element groups
    (4 i-blocks x 256 j) straight out of PSUM into bf16 SBUF tiles.
  - Attention runs as a flat lag-2 software pipeline over (j-block, i-group)
    pairs so neither the score-PSUM ring nor the output-PSUM release is ever
    on the PE's critical path; x for the next batch is loaded/transposed at
    the head of each batch and projected during its second half.
"""

import sys
import os

sys.path.insert(0, "/opt/trn_rl_repo")

import numpy as np
import ml_dtypes

B, S, C, X = 4, 2048, 16, 16
H = 8
CX = C * X  # 256
SURV = [0, 2, 3, 4, 8, 9, 10, 14]  # mv components surviving <q, ~k>
SCALE = 1.0 / np.sqrt(32.0)
NCORES = 8
SB, JB, IB = 128, 256, 128  # s-tile, j-block, i-block sizes
NST, NJB, NIB = S // SB, S // JB, S // IB  # 16, 8, 16
GRP = 4  # i-blocks per exp group
NG = NIB // GRP  # 4 groups
NV = CX + 1  # 257: v columns + denominator ones column
NWALL = 1024  # packed weight image columns (wq 256 | wk 256 | wvp 512)

_COMPILED = None


def _head_weights(h, W_qkv, b_qkv, W_out):
    """Per-head block-diagonal weight construction (bf16 device weights)."""
    f32 = np.float32
    bf16 = ml_dtypes.bfloat16
    # row h*48 + c'*3 + p  (p: 0=q, 1=k, 2=v)
    Wh = W_qkv[h * 48 : (h + 1) * 48].reshape(C, 3, C)  # [c', p, c]
    bh = b_qkv[h * 48 : (h + 1) * 48].reshape(C, 3)  # [c', p]
    Wq, Wk, Wv = Wh[:, 0], Wh[:, 1], Wh[:, 2]  # each [c', c]
    qb, kb, vb = bh[:, 0], bh[:, 1], bh[:, 2]
    Wout_h = W_out[:, np.arange(C) * H + h]  # [o, c']
    Wvp = Wout_h @ Wv  # [o, c]
    vbp = Wout_h @ vb  # [o] -> host-side add

    # x_T row layout within half: r = (c - half*8)*16 + xi
    # packed q/k column layout: d = c'*8 + si  (si indexes SURV)
    Wq_bd = np.zeros((2, 128, 128), f32)
    Wk_bd = np.zeros((2, 128, 128), f32)
    Wvp_bd = np.zeros((2, 128, CX), f32)
    for half in range(2):
        for cl in range(8):
            c = half * 8 + cl
            for si, xs in enumerate(SURV):
                r = cl * 16 + xs
                Wq_bd[half, r, np.arange(C) * 8 + si] = SCALE * Wq[:, c]
                Wk_bd[half, r, np.arange(C) * 8 + si] = Wk[:, c]
            for xi in range(16):
                r = cl * 16 + xi
                Wvp_bd[half, r, np.arange(C) * 16 + xi] = Wvp[:, c]
    qb_col = np.zeros((128, 1), f32)
    kb_col = np.zeros((128, 1), f32)
    qb_col[np.arange(C) * 8, 0] = SCALE * qb  # si=0 <-> x component 0
    kb_col[np.arange(C) * 8, 0] = kb
    # single packed bf16 weight image: [wq(2x128) | wk(2x128) | wvp(2x256)]
    # plus a tiny f32 bias pair -> two DMAs instead of eight
    W_all = np.zeros((128, NWALL), f32)
    W_all[:, 0:256] = Wq_bd.transpose(1, 0, 2).reshape(128, 256)
    W_all[:, 256:512] = Wk_bd.transpose(1, 0, 2).reshape(128, 256)
    W_all[:, 512:1024] = Wvp_bd.transpose(1, 0, 2).reshape(128, 512)
    qkb = np.concatenate([qb_col, kb_col], axis=1)  # [128, 2] f32
    return {"W_all": W_all.astype(bf16), "qkb": qkb}, vbp


def _build_program():
    import concourse.bass as bass
    import concourse.mybir as mybir
    import concourse.tile as tile
    from concourse import bacc

    f32 = mybir.dt.float32
    bf16 = mybir.dt.bfloat16
    Exp = mybir.ActivationFunctionType.Exp

    nc = bacc.Bacc("TRN2", target_bir_lowering=False, debug=False)

    x_d = nc.dram_tensor("x", [B, S, CX], bf16, kind="ExternalInput").ap()
    wall_d = nc.dram_tensor("W_all", [128, NWALL], bf16, kind="ExternalInput").ap()
    qkb_d = nc.dram_tensor("qkb", [128, 2], f32, kind="ExternalInput").ap()
    y_d = nc.dram_tensor("y", [B, S, CX], f32, kind="ExternalOutput").ap()

    with tile.TileContext(nc) as tc:
        with (
            tc.tile_pool(name="const", bufs=1) as const,
            tc.tile_pool(name="xin", bufs=8) as xin,
            tc.tile_pool(name="xT", bufs=2) as xTp,
            tc.tile_pool(name="qk", bufs=2) as qkp,
            tc.tile_pool(name="vp", bufs=2) as vpp,
            tc.tile_pool(name="es", bufs=4) as esp,
            tc.tile_pool(name="yo", bufs=3) as yop,
            tc.tile_pool(name="psm", bufs=2, space="PSUM") as psm,
            tc.tile_pool(name="pss", bufs=2, space="PSUM") as pssp,
            tc.tile_pool(name="psy", bufs=1, space="PSUM") as psyp,
        ):
            state = {}

            def load_consts():
                wall = const.tile([128, NWALL], bf16, tag="wall", name="wall")
                nc.scalar.dma_start(out=wall[:], in_=wall_d[:])
                state["wq"] = [wall[:, h * 128 : (h + 1) * 128] for h in range(2)]
                state["wk"] = [wall[:, 256 + h * 128 : 256 + (h + 1) * 128] for h in range(2)]
                state["wvp"] = [wall[:, 512 + h * CX : 512 + (h + 1) * CX] for h in range(2)]
                qkb = const.tile([128, 2], f32, tag="qkb", name="qkb")
                nc.scalar.dma_start(out=qkb[:], in_=qkb_d[:])
                state["qb_sb"] = qkb[:, 0:1]
                state["kb_sb"] = qkb[:, 1:2]

            try:
                n_rep = int(os.environ.get("BASS_REPEAT", "1"))
            except ValueError:
                n_rep = 1

            def alloc_batch(b):
                # xT layout: [c-in-half, s-tile, half, s-within-tile]
                state["xT"] = xTp.tile([128, NST, 2, SB], bf16, tag="xT", name=f"xT{b}")
                state["qp"] = qkp.tile([128, S], bf16, tag="qp", name=f"qp{b}")
                state["kp"] = qkp.tile([128, S], bf16, tag="kp", name=f"kp{b}")
                vp = vpp.tile([128, NST, NV], bf16, tag="vp", name=f"vp{b}")
                # denominator ones column (the v bias itself is added on host)
                nc.gpsimd.memset(vp[:, :, CX : CX + 1], 1.0)
                state["vp"] = vp

            def load_quad(b, q):
                """DMA a 512-token quad of x (already bf16 from the host)."""
                xt = xin.tile([128, 4, CX], bf16, tag="x", name="xt")
                src_ap = x_d[b, q * 512 : (q + 1) * 512, :].rearrange(
                    "(k p) c -> p k c", k=4, p=SB
                )
                nc.sync.dma_start(out=xt[:], in_=src_ap)
                return xt

            def transpose_quad(q, xtb):
                # out view [c, (st half), s] merges the st/half dims (contiguous)
                xT = state["xT"]
                dst = xT[:, 4 * q : 4 * q + 4].rearrange("c k h s -> c (k h) s")
                nc.sync.dma_start_transpose(
                    out=dst, in_=xtb[:].rearrange("p k c -> p (k c)")
                )

            def proj_quad(b, q):
                """Project one 512-token quad into qp/kp (bf16) and vp (bf16)."""
                wq, wk, wvp = state["wq"], state["wk"], state["wvp"]
                qb_sb, kb_sb = state["qb_sb"], state["kb_sb"]
                xT, qp, kp, vp = state["xT"], state["qp"], state["kp"], state["vp"]
                sl = slice(q * 512, (q + 1) * 512)
                stq = slice(4 * q, 4 * q + 4)
                pq = psm.tile([128, 512], f32, tag="misc", name="pq")
                nc.tensor.matmul(pq[:], wq[0], xT[:, stq, 0], start=True, stop=False)
                nc.tensor.matmul(pq[:], wq[1], xT[:, stq, 1], start=False, stop=True)
                nc.vector.tensor_scalar_add(out=qp[:, sl], in0=pq[:], scalar1=qb_sb)
                pk = psm.tile([128, 512], f32, tag="misc", name="pk")
                nc.tensor.matmul(pk[:], wk[0], xT[:, stq, 0], start=True, stop=False)
                nc.tensor.matmul(pk[:], wk[1], xT[:, stq, 1], start=False, stop=True)
                nc.vector.tensor_scalar_add(out=kp[:, sl], in0=pk[:], scalar1=kb_sb)
                for st2 in range(q * 2, q * 2 + 2):
                    pv = psm.tile([128, 512], f32, tag="misc", name="pv")
                    for u in range(2):
                        st = st2 * 2 + u
                        pvs = pv[:, u * 256 : (u + 1) * 256]
                        nc.tensor.matmul(pvs, xT[:, st, 0], wvp[0], start=True, stop=False)
                        nc.tensor.matmul(pvs, xT[:, st, 1], wvp[1], start=False, stop=True)
                        nc.vector.tensor_copy(out=vp[:, st, :CX], in_=pvs)

            def make_jb(b, qp, kp, vp, jb, tail=False):
                jsl = slice(jb * JB, (jb + 1) * JB)
                yps = [
                    psyp.tile([128, NV], f32, tag=f"yps{js}", name=f"yps{js}")
                    for js in range(2)
                ]
                es_q = {}

                def produce(g):
                    pss = pssp.tile([128, GRP, JB], f32, tag="ps_s", name="pss")
                    for gi in range(GRP):
                        ib = g * GRP + gi
                        isl = slice(ib * IB, (ib + 1) * IB)
                        nc.tensor.matmul(
                            pss[:, gi], kp[:, isl], qp[:, jsl], start=True, stop=True
                        )
                    es = esp.tile([128, GRP, JB], bf16, tag="es", name="es")
                    nc.scalar.activation(es[:], pss[:], Exp)
                    es_q[g] = es

                def consume(g):
                    # the last group runs js-major with an early PSUM release:
                    # each yps bank is copied out right after its final matmul
                    # so the next jb's accumulation can begin immediately
                    es = es_q.pop(g)
                    if g < NG - 1:
                        for gi in range(GRP):
                            ib = g * GRP + gi
                            for js in range(2):
                                nc.tensor.matmul(
                                    yps[js][:],
                                    es[:, gi, js * 128 : (js + 1) * 128],
                                    vp[:, ib],
                                    start=(ib == 0),
                                    stop=(ib == NIB - 1),
                                )
                        return
                    ysb = yop.tile([128, 2, CX], f32, tag="ysb")
                    dst_ap = y_d[b, jb * JB : (jb + 1) * JB, :].rearrange(
                        "(k p) c -> p k c", k=2, p=SB
                    )
                    for js in range(2):
                        for gi in range(GRP):
                            ib = g * GRP + gi
                            nc.tensor.matmul(
                                yps[js][:],
                                es[:, gi, js * 128 : (js + 1) * 128],
                                vp[:, ib],
                                start=(ib == 0),
                                stop=(ib == NIB - 1),
                            )
                        if tail:
                            src_n = yps[js]
                        else:
                            src_n = yop.tile([128, NV], f32, tag="yz")
                            nc.vector.tensor_copy(out=src_n[:], in_=yps[js][:])
                        rec = yop.tile([128, 1], f32, tag="rec")
                        nc.vector.reciprocal(rec[:], src_n[:, CX : CX + 1])
                        nc.vector.tensor_scalar_mul(
                            out=ysb[:, js], in0=src_n[:, :CX], scalar1=rec[:]
                        )
                        eng = nc.sync if tail else nc.gpsimd
                        eng.dma_start(out=dst_ap[:, js], in_=ysb[:, js])

                return produce, consume

            for rep in range(n_rep):
                for b in range(B):
                    first = rep == 0 and b == 0
                    last = rep == n_rep - 1 and b == B - 1
                    if first:
                        # cold start: x quad DMAs + consts first (HWDGE
                        # dispatch is the serial resource), casts split
                        # across DVE/Pool, then transposes, then the jb0
                        # attention interleaved with per-quad projections
                        alloc_batch(b)
                        xT = state["xT"]
                        for q in range(4):
                            xt = load_quad(b, q)
                            if q == 0:
                                load_consts()
                            transpose_quad(q, xt)
                        proj_quad(b, 0)
                        qp, kp, vp = state["qp"], state["kp"], state["vp"]
                        produce, consume = make_jb(b, qp, kp, vp, 0)
                        pr = cn = 0
                        for q in range(1, 4):
                            for _ in range(NG // 4):
                                produce(pr)
                                pr += 1
                            proj_quad(b, q)
                            while cn < pr - 2:
                                consume(cn)
                                cn += 1
                        while pr < NG:
                            produce(pr)
                            pr += 1
                            while cn < pr - 2:
                                consume(cn)
                                cn += 1
                        while cn < NG:
                            consume(cn)
                            cn += 1
                        jb_start = 1
                    else:
                        qp, kp, vp = state["qp"], state["kp"], state["vp"]
                        jb_start = 0

                    # flat lag-2 software pipeline over (jb, g) groups:
                    # consume trails produce by 2 groups so neither the pss
                    # ring nor the yps release is ever on the critical path
                    stream = [
                        (jb, g) for jb in range(jb_start, NJB) for g in range(NG)
                    ]
                    jbs = {}
                    pending = []

                    nb = (b + 1) % B  # next batch (wraps under BASS_REPEAT)

                    def hooks(jb):
                        if last:
                            return
                        if jb == jb_start:
                            alloc_batch(nb)
                            xts = [load_quad(nb, q) for q in range(4)]
                            for q in range(4):
                                transpose_quad(q, xts[q])
                        elif jb >= 4:
                            proj_quad(nb, jb - 4)

                    for idx, (jb, g) in enumerate(stream):
                        if g == 0:
                            hooks(jb)
                            jbs[jb] = make_jb(
                                b, qp, kp, vp, jb, tail=last and jb >= NJB - 2
                            )
                        jbs[jb][0](g)
                        pending.append((jb, g))
                        if idx >= 2:
                            cjb, cg = pending.pop(0)
                            jbs[cjb][1](cg)
                    for cjb, cg in pending:
                        jbs[cjb][1](cg)

    nc.compile()
    return nc


def kernel(x, W_qkv, b_qkv, W_out, b_out):
    global _COMPILED
    from concourse import bass_utils

    x = np.ascontiguousarray(
        np.asarray(x, dtype=np.float32).reshape(B, S, CX).astype(ml_dtypes.bfloat16)
    )
    W_qkv = np.asarray(W_qkv, dtype=np.float32)
    b_qkv = np.asarray(b_qkv, dtype=np.float32)
    W_out = np.asarray(W_out, dtype=np.float32)
    b_out = np.asarray(b_out, dtype=np.float32)

    if _COMPILED is None:
        _COMPILED = _build_program()
    nc = _COMPILED

    in_maps = []
    vbp_sum = np.zeros((C,), np.float64)
    for h in range(NCORES):
        w, vbp = _head_weights(h, W_qkv, b_qkv, W_out)
        vbp_sum += vbp.astype(np.float64)
        in_maps.append({"x": x, **w})

    try:
        trace = bool(int(os.environ.get("BASS_PROFILE", "0")))
    except ValueError:
        trace = False
    try:
        res = bass_utils.run_bass_kernel_spmd(
            nc, in_maps, core_ids=list(range(NCORES)), trace=trace
        )
    except Exception:
        # transient NRT_EXEC_UNIT_UNRECOVERABLE observed on the tunneled
        # device; a fresh attempt recovers
        import time as _time

        _time.sleep(2.0)
        res = bass_utils.run_bass_kernel_spmd(
            nc, in_maps, core_ids=list(range(NCORES)), trace=trace
        )
    if trace:
        kernel.last_exec_time_ns = res.exec_time_ns
        kernel.last_results = res

    y = np.zeros((B, S, C, X), dtype=np.float64)
    for h in range(NCORES):
        y += res.results[h]["y"].astype(np.float64).reshape(B, S, C, X)
    # attention-invariant per-head v-bias contribution + output bias,
    # both on the scalar blade only
    y[:, :, :, 0] += (vbp_sum + b_out.astype(np.float64))[None, None, :]
    return y.astype(np.float32)


# revision 44
# speedup vs baseline: 1.1883x; 1.0239x over previous
"""EquiMultiHeadAttention on 8 Trainium2 NeuronCores.

Sharding: one attention head per core (H=8, n_cores=8). Each core computes,
for all 4 batches, its head's q/k/v projections, the full SxS attention, and
that head's contribution to the output projection. The host sums the 8
partial outputs and adds the output bias (scalar blade only) plus the
attention-invariant v-bias contribution (softmax weights sum to exactly 1,
so the per-head v bias commutes with attention and is applied on the host).

Math folded into per-head host-precomputed weights:
  - q is packed to the 8 surviving mv components of the PGA inner product,
    pre-scaled by 1/sqrt(32); k packed identically -> the score matmul is a
    plain K=128 contraction.
  - The output projection (W_out columns of this head) is applied to v
    *before* attention, so the attention's second matmul directly produces
    this head's output contribution. An extra all-ones column on v yields
    the softmax denominator in the same matmul.

Whole pipeline runs in bf16 (measured end-to-end rel err ~8e-3):
  - x is cast to bf16 on the host and transposed to [(c,x), s] layout by the
    DMA xbar transpose (one instruction per 512-token quad; no PE transposes,
    no PSUM->SBUF transpose copies, no on-device casts).
  - All matmuls are bf16 (1 cycle/row on the PE).
  - Scores are exp'd by the scalar engine in 1024-element groups
    (4 i-blocks x 256 j) straight out of PSUM into bf16 SBUF tiles.
  - Attention runs as a flat lag-2 software pipeline over (j-block, i-group)
    pairs so neither the score-PSUM ring nor the output-PSUM release is ever
    on the PE's critical path; x for the next batch is loaded/transposed at
    the head of each batch and projected during its second half.
"""

import sys
import os

sys.path.insert(0, "/opt/trn_rl_repo")

import numpy as np
import ml_dtypes

B, S, C, X = 4, 2048, 16, 16
H = 8
CX = C * X  # 256
SURV = [0, 2, 3, 4, 8, 9, 10, 14]  # mv components surviving <q, ~k>
SCALE = 1.0 / np.sqrt(32.0)
NCORES = 8
SB, JB, IB = 128, 256, 128  # s-tile, j-block, i-block sizes
NST, NJB, NIB = S // SB, S // JB, S // IB  # 16, 8, 16
GRP = 4  # i-blocks per exp group
NG = NIB // GRP  # 4 groups
NV = CX + 1  # 257: v columns + denominator ones column
NWALL = 1024  # packed weight image columns (wq 256 | wk 256 | wvp 512)

_COMPILED = None


def _head_weights(h, W_qkv, b_qkv, W_out):
    """Per-head block-diagonal weight construction (bf16 device weights)."""
    f32 = np.float32
    bf16 = ml_dtypes.bfloat16
    # row h*48 + c'*3 + p  (p: 0=q, 1=k, 2=v)
    Wh = W_qkv[h * 48 : (h + 1) * 48].reshape(C, 3, C)  # [c', p, c]
    bh = b_qkv[h * 48 : (h + 1) * 48].reshape(C, 3)  # [c', p]
    Wq, Wk, Wv = Wh[:, 0], Wh[:, 1], Wh[:, 2]  # each [c', c]
    qb, kb, vb = bh[:, 0], bh[:, 1], bh[:, 2]
    Wout_h = W_out[:, np.arange(C) * H + h]  # [o, c']
    Wvp = Wout_h @ Wv  # [o, c]
    vbp = Wout_h @ vb  # [o] -> host-side add

    # x_T row layout within half: r = (c - half*8)*16 + xi
    # packed q/k column layout: d = c'*8 + si  (si indexes SURV)
    Wq_bd = np.zeros((2, 128, 128), f32)
    Wk_bd = np.zeros((2, 128, 128), f32)
    Wvp_bd = np.zeros((2, 128, CX), f32)
    for half in range(2):
        for cl in range(8):
            c = half * 8 + cl
            for si, xs in enumerate(SURV):
                r = cl * 16 + xs
                Wq_bd[half, r, np.arange(C) * 8 + si] = SCALE * Wq[:, c]
                Wk_bd[half, r, np.arange(C) * 8 + si] = Wk[:, c]
            for xi in range(16):
                r = cl * 16 + xi
                Wvp_bd[half, r, np.arange(C) * 16 + xi] = Wvp[:, c]
    qb_col = np.zeros((128, 1), f32)
    kb_col = np.zeros((128, 1), f32)
    qb_col[np.arange(C) * 8, 0] = SCALE * qb  # si=0 <-> x component 0
    kb_col[np.arange(C) * 8, 0] = kb
    # single packed bf16 weight image: [wq(2x128) | wk(2x128) | wvp(2x256)]
    # plus a tiny f32 bias pair -> two DMAs instead of eight
    W_all = np.zeros((128, NWALL), f32)
    W_all[:, 0:256] = Wq_bd.transpose(1, 0, 2).reshape(128, 256)
    W_all[:, 256:512] = Wk_bd.transpose(1, 0, 2).reshape(128, 256)
    W_all[:, 512:1024] = Wvp_bd.transpose(1, 0, 2).reshape(128, 512)
    qkb = np.concatenate([qb_col, kb_col], axis=1)  # [128, 2] f32
    return {"W_all": W_all.astype(bf16), "qkb": qkb}, vbp


def _build_program():
    import concourse.bass as bass
    import concourse.mybir as mybir
    import concourse.tile as tile
    from concourse import bacc
    from concourse.masks import make_identity

    f32 = mybir.dt.float32
    bf16 = mybir.dt.bfloat16
    Exp = mybir.ActivationFunctionType.Exp

    nc = bacc.Bacc("TRN2", target_bir_lowering=False, debug=False)

    x_d = nc.dram_tensor("x", [B, S, CX], bf16, kind="ExternalInput").ap()
    wall_d = nc.dram_tensor("W_all", [128, NWALL], bf16, kind="ExternalInput").ap()
    qkb_d = nc.dram_tensor("qkb", [128, 2], f32, kind="ExternalInput").ap()
    y_d = nc.dram_tensor("y", [B, S, CX], f32, kind="ExternalOutput").ap()

    with tile.TileContext(nc) as tc:
        with (
            tc.tile_pool(name="const", bufs=1) as const,
            tc.tile_pool(name="xin", bufs=8) as xin,
            tc.tile_pool(name="xT", bufs=2) as xTp,
            tc.tile_pool(name="qk", bufs=2) as qkp,
            tc.tile_pool(name="vp", bufs=2) as vpp,
            tc.tile_pool(name="es", bufs=4) as esp,
            tc.tile_pool(name="yo", bufs=3) as yop,
            tc.tile_pool(name="psm", bufs=2, space="PSUM") as psm,
            tc.tile_pool(name="pss", bufs=2, space="PSUM") as pssp,
            tc.tile_pool(name="psy", bufs=1, space="PSUM") as psyp,
        ):
            state = {}

            def load_consts():
                wall = const.tile([128, NWALL], bf16, tag="wall", name="wall")
                nc.scalar.dma_start(out=wall[:], in_=wall_d[:])
                state["wq"] = [wall[:, h * 128 : (h + 1) * 128] for h in range(2)]
                state["wk"] = [wall[:, 256 + h * 128 : 256 + (h + 1) * 128] for h in range(2)]
                state["wvp"] = [wall[:, 512 + h * CX : 512 + (h + 1) * CX] for h in range(2)]
                qkb = const.tile([128, 2], f32, tag="qkb", name="qkb")
                nc.scalar.dma_start(out=qkb[:], in_=qkb_d[:])
                state["qb_sb"] = qkb[:, 0:1]
                state["kb_sb"] = qkb[:, 1:2]

            try:
                n_rep = int(os.environ.get("BASS_REPEAT", "1"))
            except ValueError:
                n_rep = 1

            def alloc_batch(b):
                # xT layout: [c-in-half, s-tile, half, s-within-tile]
                state["xT"] = xTp.tile([128, NST, 2, SB], bf16, tag="xT", name=f"xT{b}")
                state["qp"] = qkp.tile([128, S], bf16, tag="qp", name=f"qp{b}")
                state["kp"] = qkp.tile([128, S], bf16, tag="kp", name=f"kp{b}")
                vp = vpp.tile([128, NST, NV], bf16, tag="vp", name=f"vp{b}")
                # denominator ones column (the v bias itself is added on host)
                nc.gpsimd.memset(vp[:, :, CX : CX + 1], 1.0)
                state["vp"] = vp

            def load_quad(b, q):
                """DMA a 512-token quad of x (already bf16 from the host)."""
                xt = xin.tile([128, 4, CX], bf16, tag="x", name="xt")
                src_ap = x_d[b, q * 512 : (q + 1) * 512, :].rearrange(
                    "(k p) c -> p k c", k=4, p=SB
                )
                nc.sync.dma_start(out=xt[:], in_=src_ap)
                return xt

            def transpose_quad(q, xtb, eng=None):
                # out view [c, (st half), s] merges the st/half dims (contiguous)
                xT = state["xT"]
                dst = xT[:, 4 * q : 4 * q + 4].rearrange("c k h s -> c (k h) s")
                (eng or nc.sync).dma_start_transpose(
                    out=dst, in_=xtb[:].rearrange("p k c -> p (k c)")
                )

            def proj_quad(b, q):
                """Project one 512-token quad into qp/kp (bf16) and vp (bf16)."""
                wq, wk, wvp = state["wq"], state["wk"], state["wvp"]
                qb_sb, kb_sb = state["qb_sb"], state["kb_sb"]
                xT, qp, kp, vp = state["xT"], state["qp"], state["kp"], state["vp"]
                sl = slice(q * 512, (q + 1) * 512)
                stq = slice(4 * q, 4 * q + 4)
                pq = psm.tile([128, 512], f32, tag="misc", name="pq")
                nc.tensor.matmul(pq[:], wq[0], xT[:, stq, 0], start=True, stop=False)
                nc.tensor.matmul(pq[:], wq[1], xT[:, stq, 1], start=False, stop=True)
                nc.vector.tensor_scalar_add(out=qp[:, sl], in0=pq[:], scalar1=qb_sb)
                pk = psm.tile([128, 512], f32, tag="misc", name="pk")
                nc.tensor.matmul(pk[:], wk[0], xT[:, stq, 0], start=True, stop=False)
                nc.tensor.matmul(pk[:], wk[1], xT[:, stq, 1], start=False, stop=True)
                nc.vector.tensor_scalar_add(out=kp[:, sl], in0=pk[:], scalar1=kb_sb)
                for st2 in range(q * 2, q * 2 + 2):
                    pv = psm.tile([128, 512], f32, tag="misc", name="pv")
                    for u in range(2):
                        st = st2 * 2 + u
                        pvs = pv[:, u * 256 : (u + 1) * 256]
                        nc.tensor.matmul(pvs, xT[:, st, 0], wvp[0], start=True, stop=False)
                        nc.tensor.matmul(pvs, xT[:, st, 1], wvp[1], start=False, stop=True)
                        nc.vector.tensor_copy(out=vp[:, st, :CX], in_=pvs)

            def make_jb(b, qp, kp, vp, jb, tail=False):
                jsl = slice(jb * JB, (jb + 1) * JB)
                yps = [
                    psyp.tile([128, NV], f32, tag=f"yps{js}", name=f"yps{js}")
                    for js in range(2)
                ]
                es_q = {}

                def produce(g):
                    pss = pssp.tile([128, GRP, JB], f32, tag="ps_s", name="pss")
                    for gi in range(GRP):
                        ib = g * GRP + gi
                        isl = slice(ib * IB, (ib + 1) * IB)
                        nc.tensor.matmul(
                            pss[:, gi], kp[:, isl], qp[:, jsl], start=True, stop=True
                        )
                    es = esp.tile([128, GRP, JB], bf16, tag="es", name="es")
                    nc.scalar.activation(es[:], pss[:], Exp)
                    es_q[g] = es

                def consume(g):
                    # the last group runs js-major with an early PSUM release:
                    # each yps bank is copied out right after its final matmul
                    # so the next jb's accumulation can begin immediately
                    es = es_q.pop(g)
                    if g < NG - 1:
                        for gi in range(GRP):
                            ib = g * GRP + gi
                            for js in range(2):
                                nc.tensor.matmul(
                                    yps[js][:],
                                    es[:, gi, js * 128 : (js + 1) * 128],
                                    vp[:, ib],
                                    start=(ib == 0),
                                    stop=(ib == NIB - 1),
                                )
                        return
                    ysb = yop.tile([128, 2, CX], f32, tag="ysb")
                    dst_ap = y_d[b, jb * JB : (jb + 1) * JB, :].rearrange(
                        "(k p) c -> p k c", k=2, p=SB
                    )
                    for js in range(2):
                        for gi in range(GRP):
                            ib = g * GRP + gi
                            nc.tensor.matmul(
                                yps[js][:],
                                es[:, gi, js * 128 : (js + 1) * 128],
                                vp[:, ib],
                                start=(ib == 0),
                                stop=(ib == NIB - 1),
                            )
                        if tail:
                            src_n = yps[js]
                        else:
                            src_n = yop.tile([128, NV], f32, tag="yz")
                            nc.vector.tensor_copy(out=src_n[:], in_=yps[js][:])
                        rec = yop.tile([128, 1], f32, tag="rec")
                        nc.vector.reciprocal(rec[:], src_n[:, CX : CX + 1])
                        nc.vector.tensor_scalar_mul(
                            out=ysb[:, js], in0=src_n[:, :CX], scalar1=rec[:]
                        )
                        eng = nc.sync if tail else nc.gpsimd
                        eng.dma_start(out=dst_ap[:, js], in_=ysb[:, js])

                return produce, consume

            for rep in range(n_rep):
                for b in range(B):
                    first = rep == 0 and b == 0
                    last = rep == n_rep - 1 and b == B - 1
                    if first:
                        # cold start: x quad DMAs + consts first (HWDGE
                        # dispatch is the serial resource), casts split
                        # across DVE/Pool, then transposes, then the jb0
                        # attention interleaved with per-quad projections
                        alloc_batch(b)
                        xT = state["xT"]
                        ident = const.tile([128, 128], bf16, tag="ident")
                        make_identity(nc, ident[:])
                        xts = []
                        for q in range(4):
                            xts.append(load_quad(b, q))
                            if q == 0:
                                load_consts()
                        # quads 0-1: PE transposes (PE is idle during the
                        # ramp; scratch borrows the idle yps PSUM banks),
                        # quads 2-3: DMA xbar in parallel
                        transpose_quad(2, xts[2])
                        transpose_quad(3, xts[3])
                        xT = state["xT"]
                        for q in range(2):
                            for half in range(2):
                                ptf = psyp.tile(
                                    [128, NV], f32, tag=f"yps{half}", name=f"pt{q}{half}"
                                )
                                pt = ptf[:, 0:256].bitcast(bf16)
                                for k in range(4):
                                    nc.tensor.transpose(
                                        pt[:, k * 128 : (k + 1) * 128],
                                        xts[q][:, k, half * 128 : (half + 1) * 128],
                                        ident[:],
                                    )
                                nc.vector.tensor_copy(
                                    out=xT[:, 4 * q : 4 * q + 4, half, :],
                                    in_=pt.rearrange("p (k s) -> p k s", k=4),
                                )
                        proj_quad(b, 0)
                        qp, kp, vp = state["qp"], state["kp"], state["vp"]
                        produce, consume = make_jb(b, qp, kp, vp, 0)
                        pr = cn = 0
                        for q in range(1, 4):
                            for _ in range(NG // 4):
                                produce(pr)
                                pr += 1
                            proj_quad(b, q)
                            while cn < pr - 2:
                                consume(cn)
                                cn += 1
                        while pr < NG:
                            produce(pr)
                            pr += 1
                            while cn < pr - 2:
                                consume(cn)
                                cn += 1
                        while cn < NG:
                            consume(cn)
                            cn += 1
                        jb_start = 1
                    else:
                        qp, kp, vp = state["qp"], state["kp"], state["vp"]
                        jb_start = 0

                    # flat lag-2 software pipeline over (jb, g) groups:
                    # consume trails produce by 2 groups so neither the pss
                    # ring nor the yps release is ever on the critical path
                    stream = [
                        (jb, g) for jb in range(jb_start, NJB) for g in range(NG)
                    ]
                    jbs = {}
                    pending = []

                    nb = (b + 1) % B  # next batch (wraps under BASS_REPEAT)

                    def hooks(jb):
                        if last:
                            return
                        if jb == jb_start:
                            alloc_batch(nb)
                            xts = [load_quad(nb, q) for q in range(4)]
                            for q in range(4):
                                transpose_quad(q, xts[q])
                        elif jb >= 4:
                            proj_quad(nb, jb - 4)

                    for idx, (jb, g) in enumerate(stream):
                        if g == 0:
                            hooks(jb)
                            jbs[jb] = make_jb(
                                b, qp, kp, vp, jb, tail=last and jb >= NJB - 2
                            )
                        jbs[jb][0](g)
                        pending.append((jb, g))
                        if idx >= 2:
                            cjb, cg = pending.pop(0)
                            jbs[cjb][1](cg)
                    for cjb, cg in pending:
                        jbs[cjb][1](cg)

    nc.compile()
    return nc


def kernel(x, W_qkv, b_qkv, W_out, b_out):
    global _COMPILED
    from concourse import bass_utils

    x = np.ascontiguousarray(
        np.asarray(x, dtype=np.float32).reshape(B, S, CX).astype(ml_dtypes.bfloat16)
    )
    W_qkv = np.asarray(W_qkv, dtype=np.float32)
    b_qkv = np.asarray(b_qkv, dtype=np.float32)
    W_out = np.asarray(W_out, dtype=np.float32)
    b_out = np.asarray(b_out, dtype=np.float32)

    if _COMPILED is None:
        _COMPILED = _build_program()
    nc = _COMPILED

    in_maps = []
    vbp_sum = np.zeros((C,), np.float64)
    for h in range(NCORES):
        w, vbp = _head_weights(h, W_qkv, b_qkv, W_out)
        vbp_sum += vbp.astype(np.float64)
        in_maps.append({"x": x, **w})

    try:
        trace = bool(int(os.environ.get("BASS_PROFILE", "0")))
    except ValueError:
        trace = False
    try:
        res = bass_utils.run_bass_kernel_spmd(
            nc, in_maps, core_ids=list(range(NCORES)), trace=trace
        )
    except Exception:
        # transient NRT_EXEC_UNIT_UNRECOVERABLE observed on the tunneled
        # device; a fresh attempt recovers
        import time as _time

        _time.sleep(2.0)
        res = bass_utils.run_bass_kernel_spmd(
            nc, in_maps, core_ids=list(range(NCORES)), trace=trace
        )
    if trace:
        kernel.last_exec_time_ns = res.exec_time_ns
        kernel.last_results = res

    y = np.zeros((B, S, C, X), dtype=np.float64)
    for h in range(NCORES):
        y += res.results[h]["y"].astype(np.float64).reshape(B, S, C, X)
    # attention-invariant per-head v-bias contribution + output bias,
    # both on the scalar blade only
    y[:, :, :, 0] += (vbp_sum + b_out.astype(np.float64))[None, None, :]
    return y.astype(np.float32)
